# revision 1
# baseline (speedup 1.0000x reference)
"""Trainium2 Bass kernel for 3D conv-attention layer.

Reference (per (b,h,w) "site", D=32 positions, S=32 features):
  k,q,v = 1x1 conv of x [B,C,D,H,W] -> [B,S,D,H,W]
  scoresT[j,i] = sum_s q[s,j] k[s,i] / sqrt(S)   (per site)
  aT = softmax over i  (free dim of scoresT)
  o[s,j] = sum_i v[s,i] a[i,j];   y = x + Wo @ o + bo

Sharding: data-parallel over H across 8 cores.

Per-core strategy (per (b,h) chunk = 64 sites; halves of 32 sites):
  - Grid projections (tile_position col groups): K=64, M=32, N=256 matmuls
    place per-site [S,32] k/q/v tiles on distinct 32-partition blocks so that
    16 sites' attention matmuls run concurrently in the 128x128 PE array.
  - scoresT via 16 concurrent 32x32x32 matmuls; softmax over the free dim
    (exp without max-subtraction: |scores| <~ 7, exact-safe in fp32).
  - v->vT and aT->a via DVE 32x32 block transposes.
  - o via 16 concurrent matmuls -> [s, d] grid; output projection via 8
    packed matmuls (N=256); residual + out-proj bias + re-layout fused into
    per-row-group tensor_tensor ops reading PSUM directly.
  - All matmuls fp32 (exact): fp32r is ~2e-4 lossy on HW and only legal at
    tile_position column 0, which breaks the grid layout.
  - HW constraints honored: one sem-wait per instruction (Bacc event sems),
    and concurrent tile_position matmuls sharing a column group must write
    distinct PSUM banks (device crash otherwise).
"""

import math
from contextlib import ExitStack

import numpy as np

import concourse.bass as bass
import concourse.mybir as mybir
from concourse import bacc
import concourse.tile as tile
from concourse.bass_utils import run_bass_kernel_spmd

B, C, D, H, W = 4, 64, 32, 64, 64
S = C // 2  # 32
NCORES = 8
HS = H // NCORES
F32 = mybir.dt.float32
FR = mybir.dt.float32r

INV_SQRT_S = 1.0 / math.sqrt(S)


def mkap(base, part0, pcount, foff, fdims):
    """AP at partition block [part0, part0+pcount) of a tile, free offset foff,
    free dims [(step, count), ...] in the tile's flat free space."""
    full = base[...] if not isinstance(base, bass.AP) else base
    pstride = full.ap[0][0]
    return bass.AP(tensor=full.tensor,
                   offset=full.offset + part0 * pstride + foff,
                   ap=[[pstride, pcount]] + [list(d) for d in fdims])


def dap(handle, offset, dims):
    """Raw AP on a DRAM tensor: dims are [[step, count], ...] in elements."""
    full = handle[...]
    return bass.AP(tensor=full.tensor, offset=offset,
                   ap=[list(d) for d in dims])


def build_program(attn_dt=F32, proj_dt=FR):
    nc = bacc.Bacc()
    x_d = nc.declare_dram_parameter("x", [B, C, D, HS, W], F32, isOutput=False)
    # host-precomputed constant layouts (see make_in_maps)
    wk_d = nc.declare_dram_parameter("wkT", [C, S], F32, isOutput=False)
    wq_d = nc.declare_dram_parameter("wqT", [C, S], F32, isOutput=False)
    wv_d = nc.declare_dram_parameter("wvT", [C, S], F32, isOutput=False)
    wo_d = nc.declare_dram_parameter("woTr", [4 * S, C], F32, isOutput=False)
    bk_d = nc.declare_dram_parameter("bkr", [128, 1], F32, isOutput=False)
    bq_d = nc.declare_dram_parameter("bqr", [128, 1], F32, isOutput=False)
    bv_d = nc.declare_dram_parameter("bvr", [128, 1], F32, isOutput=False)
    bo_d = nc.declare_dram_parameter("boc", [C, 1], F32, isOutput=False)
    y_d = nc.declare_dram_parameter("y", [B, C, D, HS, W], F32, isOutput=True)

    def mm_dt(apx, dt):
        return apx.bitcast(dt) if dt != F32 else apx

    with tile.TileContext(nc) as tc, ExitStack() as ctx:
        const = ctx.enter_context(tc.tile_pool(name="const", bufs=1))
        xp = ctx.enter_context(tc.tile_pool(name="xp", bufs=3))
        att_ps = ctx.enter_context(tc.tile_pool(name="att_ps", bufs=1, space="PSUM"))
        sb = ctx.enter_context(tc.tile_pool(name="sb", bufs=3))
        outp = ctx.enter_context(tc.tile_pool(name="outp", bufs=2))

        # ---- constants (host-prelayouted; fp32r rounding copies on DVE) ----
        wkT_s = const.tile([C, S], F32, tag="wks")
        wqT_s = const.tile([C, S], F32, tag="wqs")
        wvT_s = const.tile([C, S], F32, tag="wvs")
        wkT = const.tile([C, S], proj_dt, tag="wk")
        wqT = const.tile([C, S], proj_dt, tag="wq")
        wvT = const.tile([C, S], proj_dt, tag="wv")
        for wt, ws, wd in ((wkT, wkT_s, wk_d), (wqT, wqT_s, wq_d),
                           (wvT, wvT_s, wv_d)):
            nc.sync.dma_start(out=ws[:, :], in_=wd[:, :])
            nc.vector.tensor_copy(out=wt[:, :], in_=ws[:, :])
        woT_s = const.tile([4 * S, C], F32, tag="wos")
        woT = const.tile([4 * S, C], proj_dt, tag="wo")
        nc.sync.dma_start(out=woT_s[:, :], in_=wo_d[:, :])
        nc.vector.tensor_copy(out=woT[:, :], in_=woT_s[:, :])
        bk_t = const.tile([128, 1], F32, tag="bk")
        bq_t = const.tile([128, 1], F32, tag="bq")
        bv_t = const.tile([128, 1], F32, tag="bv")
        for bt, bd in ((bk_t, bk_d), (bq_t, bq_d), (bv_t, bv_d)):
            nc.sync.dma_start(out=bt[:, :], in_=bd[:, :])
        bo_c = const.tile([C, 1], F32, tag="bo")
        nc.sync.dma_start(out=bo_c[:, :], in_=bo_d[:, :])

        for b in range(B):
            for h in range(HS):
                x_sb = xp.tile([C, D, W], F32, tag="x")
                # split by w-half: half 0's projections start after only
                # half the load; two DMA queues run in parallel per chunk
                nc.sync.dma_start(out=x_sb[:, :, 0:32],
                                  in_=x_d[b, :, :, h, 0:32])
                nc.sync.dma_start(out=x_sb[:, :, 32:64],
                                  in_=x_d[b, :, :, h, 32:64])
                y_sb = xp.tile([C, D, W], F32, tag="y")
                # x + bo precomputed (residual + out-proj bias in one term)
                xb_sb = xp.tile([C, D, W], F32, tag="xb")
                nc.gpsimd.tensor_scalar_add(xb_sb[:, :, :], x_sb[:, :, :],
                                            bo_c[:, :])
                if proj_dt != F32:
                    # x rounded to proj_dt for fp32r (ACT is idle)
                    x_r = xp.tile([C, D, W], proj_dt, tag="xr")
                    nc.scalar.activation(x_r[:, :, :], x_sb[:, :, :],
                                         mybir.ActivationFunctionType.Copy)
                else:
                    x_r = x_sb

                for half in range(2):
                    wb = 32 * half

                    # ---- projections into grid layouts ----
                    kg = att_ps.tile([128, 256], F32, tag="T0")
                    qg = att_ps.tile([128, 256], F32, tag="T1")
                    vg = att_ps.tile([128, 256], F32, tag="T2")
                    for r in range(4):
                        # sites idx%4==r: w = wb+r+4u, u=0..7; col = 32u+d
                        rhs = mkap(x_r, 0, C, wb + r, [[4, 8], [W, D]])
                        nc.tensor.matmul(kg[32 * r:32 * r + 32, :],
                                         wkT[:, :], rhs,
                                         start=True, stop=True,
                                         tile_position=(0, 32 * r))
                        nc.tensor.matmul(qg[32 * r:32 * r + 32, :],
                                         wqT[:, :], rhs,
                                         start=True, stop=True,
                                         tile_position=(0, 32 * r))
                    for c in range(4):
                        # sites (idx%16)//4==c: w = wb+16*s16+4c+jl
                        # col = 128*s16 + 32*jl + d
                        rhs = mkap(x_r, 0, C, wb + 4 * c,
                                   [[16, 2], [1, 4], [W, D]])
                        nc.tensor.matmul(vg[32 * c:32 * c + 32, :],
                                         wvT[:, :], rhs,
                                         start=True, stop=True,
                                         tile_position=(0, 32 * c))

                    # ---- PSUM -> SBUF with bias ----
                    k_sb = sb.tile([128, 256], F32, tag="k")
                    q_sb = sb.tile([128, 256], F32, tag="q")
                    v_sb = sb.tile([128, 256], F32, tag="v")
                    # k on DVE in parallel with q on ACT: scores need both,
                    # so splitting the drains across engines shortens the
                    # PE-critical path each half.
                    nc.vector.tensor_scalar_add(k_sb[:, :], kg[:, :], bk_t[:, :])
                    nc.scalar.activation(q_sb[:, :], qg[:, :],
                                         mybir.ActivationFunctionType.Identity,
                                         bias=bq_t[:, :])
                    nc.vector.tensor_scalar_add(v_sb[:, :], vg[:, :], bv_t[:, :])

                    vT_sb = sb.tile([128, 256], F32, tag="vT")
                    nc.vector.transpose(vT_sb[:, :], v_sb[:, :])

                    # PSUM banks: scores MMs sharing a column group from
                    # different row groups must land in different banks
                    # (HW crash otherwise) -> one bank tile per row group.
                    obank = []
                    for c in range(4):
                        ot = att_ps.tile([128, 64], F32, tag=f"T{c}")
                        obank.append(ot)

                    for s16 in range(2):
                        fo = 128 * s16
                        scb = []
                        for r in range(4):
                            st = att_ps.tile([128, 32], F32, tag=f"U{r}")
                            scb.append(st)
                        for j in range(16):
                            r, c = j % 4, j // 4
                            col = fo + 32 * c
                            nc.tensor.matmul(
                                scb[r][32 * c:32 * c + 32, 0:32],
                                mm_dt(q_sb[32 * r:32 * r + 32, col:col + 32], attn_dt),
                                mm_dt(k_sb[32 * r:32 * r + 32, col:col + 32], attn_dt),
                                start=True, stop=True,
                                tile_position=(32 * r, 32 * c))

                        # softmax over free dim
                        e_sb = sb.tile([128, 128], F32, tag="e")
                        for r in range(4):
                            nc.scalar.activation(e_sb[:, 32 * r:32 * r + 32],
                                                 scb[r][:, 0:32],
                                                 mybir.ActivationFunctionType.Exp,
                                                 scale=INV_SQRT_S)
                        den = sb.tile([128, 4], F32, tag="den")
                        nc.vector.reduce_sum(
                            out=den[:, :],
                            in_=mkap(e_sb, 0, 128, 0, [[32, 4], [1, 32]]),
                            axis=mybir.AxisListType.X)
                        rcp = sb.tile([128, 4], F32, tag="rcp")
                        nc.vector.reciprocal(rcp[:, :], den[:, :])
                        aT_sb = sb.tile([128, 128], F32, tag="aT")
                        nc.vector.tensor_tensor(
                            out=mkap(aT_sb, 0, 128, 0, [[32, 4], [1, 32]]),
                            in0=mkap(e_sb, 0, 128, 0, [[32, 4], [1, 32]]),
                            in1=mkap(rcp, 0, 128, 0, [[1, 4], [0, 32]]),
                            op=mybir.AluOpType.mult)
                        a_sb = sb.tile([128, 128], F32, tag="a")
                        nc.vector.transpose(a_sb[:, :], aT_sb[:, :])

                        # o-MM (site r,c): row group c, col group r ->
                        # bank by row group c; free offset 32*s16
                        for j in range(16):
                            r, c = j % 4, j // 4
                            nc.tensor.matmul(
                                obank[c][32 * r:32 * r + 32,
                                         32 * s16:32 * s16 + 32],
                                mm_dt(vT_sb[32 * c:32 * c + 32,
                                            fo + 32 * r:fo + 32 * r + 32], attn_dt),
                                mm_dt(a_sb[32 * c:32 * c + 32,
                                           32 * r:32 * r + 32], attn_dt),
                                start=True, stop=True,
                                tile_position=(32 * c, 32 * r))

                    # gather o banks -> o_sb [128, 256]: block (r, 128*s16+32*c)
                    o_sb = sb.tile([128, 256], proj_dt, tag="osb")
                    for c in range(4):
                        nc.scalar.activation(
                            mkap(o_sb, 0, 128, 32 * c, [[128, 2], [1, 32]]),
                            mkap(obank[c], 0, 128, 0, [[32, 2], [1, 32]]),
                            mybir.ActivationFunctionType.Copy)

                    # ---- output projection: 8 matmuls N=256 ----
                    # per-row-group banks (reuse U tags; scb dead by now)
                    opb = []
                    for r in range(4):
                        pt = att_ps.tile([C, 256], F32, tag=f"U{r}")
                        opb.append(pt)
                    for r in range(4):
                        for bh in range(2):
                            nc.tensor.matmul(
                                opb[r][32 * bh:32 * bh + 32, 0:256],
                                woT[32 * r:32 * r + 32, 32 * bh:32 * bh + 32],
                                o_sb[32 * r:32 * r + 32, :],
                                start=True, stop=True,
                                tile_position=(32 * r, 32 * bh))

                    # residual + re-layout: value (c_ch, w=wb+16s16+4c+r, dj)
                    # at opb[r] partition c_ch, free 128*s16 + 32*c + dj.
                    for r in range(4):
                        fdims_o = [[128, 2], [32, 4], [1, 32]]
                        fdims_x = [[16, 2], [4, 4], [W, D]]
                        in0 = mkap(opb[r], 0, C, 0, fdims_o)
                        x_in = mkap(xb_sb, 0, C, wb + r, fdims_x)
                        y_out = mkap(y_sb, 0, C, wb + r, fdims_x)
                        nc.vector.tensor_tensor(out=y_out, in0=in0, in1=x_in,
                                                op=mybir.AluOpType.add)

                nc.sync.dma_start(out=y_d[b, :, :, h, :], in_=y_sb[:, :, :])

    nc.finalize()
    return nc


_NC_CACHE = {}


def get_nc(key=("f32", "f32")):
    if key not in _NC_CACHE:
        dts = {"f32": F32, "fr": FR}
        _NC_CACHE[key] = build_program(attn_dt=dts[key[0]], proj_dt=dts[key[1]])
    return _NC_CACHE[key]


def make_in_maps(x, Wk, bk, Wq, bq, Wv, bv, Wo, bo):
    x = np.ascontiguousarray(np.asarray(x, dtype=np.float32))
    f = np.float32
    rep4 = lambda v: np.tile(np.asarray(v, f).reshape(-1), 4)[:, None]
    consts = {
        "wkT": np.ascontiguousarray(np.asarray(Wk, f).T),
        "wqT": np.ascontiguousarray(np.asarray(Wq, f).T),
        "wvT": np.ascontiguousarray(np.asarray(Wv, f).T),
        "woTr": np.ascontiguousarray(np.tile(np.asarray(Wo, f).T, (4, 1))),
        "bkr": np.ascontiguousarray(rep4(bk)),
        "bqr": np.ascontiguousarray(rep4(bq)),
        "bvr": np.ascontiguousarray(rep4(bv)),
        "boc": np.ascontiguousarray(np.asarray(bo, f)[:, None]),
    }
    in_maps = []
    for i in range(NCORES):
        m = {"x": np.ascontiguousarray(x[:, :, :, i * HS:(i + 1) * HS, :])}
        m.update(consts)
        in_maps.append(m)
    return in_maps


def gather(results):
    out = np.empty((B, C, D, H, W), dtype=np.float32)
    for i in range(NCORES):
        out[:, :, :, i * HS:(i + 1) * HS, :] = results[i]["y"]
    return out


def kernel(x, Wk, bk, Wq, bq, Wv, bv, Wo, bo):
    nc = get_nc()
    in_maps = make_in_maps(x, Wk, bk, Wq, bq, Wv, bv, Wo, bo)
    res = run_bass_kernel_spmd(nc, in_maps, core_ids=list(range(NCORES)))
    return gather(res.results)



# revision 48
# speedup vs baseline: 3.6174x; 3.6174x over previous
"""Trainium2 Bass kernel for 3D conv-attention layer.

Reference (per (b,h,w) "site", D=32 positions, S=32 features):
  k,q,v = 1x1 conv of x [B,C,D,H,W] -> [B,S,D,H,W]
  scoresT[j,i] = sum_s q[s,j] k[s,i] / sqrt(S)   (per site)
  aT = softmax over i  (free dim of scoresT)
  o[s,j] = sum_i v[s,i] a[i,j];   y = x + Wo @ o + bo

Sharding: data-parallel over H across 8 cores (HS=8 rows each).

Per-core v3 design (cost-model driven):
  - Chunks of (b, h-pair): x/y DMA'd as [C, D, 2, W] so contiguous runs are
    2*W*4 = 512B (avoids the <512B 2x DMA latency penalty).
  - Software pipeline at chunk level: proj halves of chunk N interleave with
    attention halves of chunk N-1 in emission order, so the PE always has
    independent work while a half's softmax chain (ACT/DVE) is in flight.
  - All matmuls bf16 (1 PE cycle/row) except nothing in fp32 (4 cycles/row).
    x cast to bf16 for free via gpsimd software-DGE SBUF->SBUF DMA.
  - Per-site 32x32 attention matmuls into shared PSUM tiles so softmax runs
    as single whole-tile instructions ([128,256] per half of 32 sites).
  - PSUM: 8 banks = kq[128,512]x1 + vg[128,256]x1 + attn(sc|og)[128,512]x2
    + (op0,op1)[64,512]x2.
  - Residual y = op + (x+bo) as two 3-free-dim DVE adds per half; x+bo
    computed in place on the otherwise-idle GPSIMD (Pool).
  - Output y DMA'd from the ACT HWDGE queue so it never blocks input loads
    on the SP queue.
"""

import math
from contextlib import ExitStack

import numpy as np

import concourse.bass as bass
import concourse.mybir as mybir
from concourse import bacc
import concourse.tile as tile
from concourse.bass_utils import run_bass_kernel_spmd

B, C, D, H, W = 4, 64, 32, 64, 64
S = C // 2  # 32
NCORES = 8
HS = H // NCORES
HP = HS // 2  # h-pairs (chunks) per (b,) => 4
F32 = mybir.dt.float32
FR = mybir.dt.float32r
BF16 = mybir.dt.bfloat16

INV_SQRT_S = 1.0 / math.sqrt(S)


def mkap(base, part0, pcount, foff, fdims):
    """AP at partition block [part0, part0+pcount) of a tile, free offset foff,
    free dims [(step, count), ...] in the tile's flat free space."""
    full = base[...] if not isinstance(base, bass.AP) else base
    pstride = full.ap[0][0]
    return bass.AP(tensor=full.tensor,
                   offset=full.offset + part0 * pstride + foff,
                   ap=[[pstride, pcount]] + [list(d) for d in fdims])


def build_program(kq_bufs=2, vt_bufs=1, at_bufs=3, op_bufs=1):
    nc = bacc.Bacc()
    x_d = nc.declare_dram_parameter("x", [B, C, D, HS, W], F32, isOutput=False)
    wk_d = nc.declare_dram_parameter("wkT", [C, S], F32, isOutput=False)
    wq_d = nc.declare_dram_parameter("wqT", [C, S], F32, isOutput=False)
    wv_d = nc.declare_dram_parameter("wvT", [C, S], F32, isOutput=False)
    wo_d = nc.declare_dram_parameter("woTr", [4 * S, C], F32, isOutput=False)
    bk_d = nc.declare_dram_parameter("bkr", [128, 1], F32, isOutput=False)
    bq_d = nc.declare_dram_parameter("bqr", [128, 1], F32, isOutput=False)
    bv_d = nc.declare_dram_parameter("bvr", [128, 1], F32, isOutput=False)
    bo_d = nc.declare_dram_parameter("boc", [C, 1], F32, isOutput=False)
    y_d = nc.declare_dram_parameter("y", [B, C, D, HS, W], F32, isOutput=True)

    with tile.TileContext(nc) as tc, ExitStack() as ctx:
        const = ctx.enter_context(tc.tile_pool(name="const", bufs=1))
        xp = ctx.enter_context(tc.tile_pool(name="xp", bufs=3))
        yp = ctx.enter_context(tc.tile_pool(name="yp", bufs=3))
        kg_ps = ctx.enter_context(tc.tile_pool(name="kg_ps", bufs=1, space="PSUM"))
        qg_ps = ctx.enter_context(tc.tile_pool(name="qg_ps", bufs=1, space="PSUM"))
        vt_ps = ctx.enter_context(tc.tile_pool(name="vt_ps", bufs=1, space="PSUM"))
        sc_ps = ctx.enter_context(tc.tile_pool(name="sc_ps", bufs=2, space="PSUM"))
        og_ps = ctx.enter_context(tc.tile_pool(name="og_ps", bufs=1, space="PSUM"))
        op_ps = ctx.enter_context(tc.tile_pool(name="op_ps", bufs=1, space="PSUM"))
        sb = ctx.enter_context(tc.tile_pool(name="sb", bufs=6))

        CH = [(b, hp) for b in range(B) for hp in range(HP)]

        first_ld = {}

        def emit_first_load():
            b, hp = CH[0]
            h0 = 2 * hp
            x_sb = xp.tile([C, D, 2, W], F32, tag="x")
            nc.sync.dma_start(out=x_sb[:, :, :, :],
                              in_=x_d[b, :, :, h0:h0 + 2, :])
            x_bf = xp.tile([C, D, 2, W], BF16, tag="xbf")
            nc.scalar.activation(x_bf[:, :, :, :], x_sb[:, :, :, :],
                                 mybir.ActivationFunctionType.Copy)
            first_ld.update({"x": x_sb, "xbf": x_bf,
                             "half": [None] * 4, "ci": 0})

        emit_first_load()

        # ---- constants (weights cast to bf16 on DVE once) ----
        wkT_s = const.tile([C, S], F32, tag="wks")
        wqT_s = const.tile([C, S], F32, tag="wqs")
        wvT_s = const.tile([C, S], F32, tag="wvs")
        wkT = const.tile([C, S], BF16, tag="wk")
        wqT = const.tile([C, S], BF16, tag="wq")
        wvT = const.tile([C, S], BF16, tag="wv")
        for wt, ws, wd in ((wkT, wkT_s, wk_d), (wqT, wqT_s, wq_d),
                           (wvT, wvT_s, wv_d)):
            nc.sync.dma_start(out=ws[:, :], in_=wd[:, :])
            nc.vector.tensor_copy(out=wt[:, :], in_=ws[:, :])
        # out-proj weights 4x-tiled on partitions (lhsT base must match the
        # rhs partition base 32r), cast to bf16
        woT_s = const.tile([4 * S, C], F32, tag="wos")
        woT = const.tile([4 * S, C], BF16, tag="wo")
        nc.sync.dma_start(out=woT_s[:, :], in_=wo_d[:, :])
        nc.vector.tensor_copy(out=woT[:, :], in_=woT_s[:, :])
        bk_t = const.tile([128, 1], F32, tag="bk")
        bq_t = const.tile([128, 1], F32, tag="bq")
        bv_t = const.tile([128, 1], F32, tag="bv")
        for bt, bd in ((bk_t, bk_d), (bq_t, bq_d), (bv_t, bv_d)):
            nc.sync.dma_start(out=bt[:, :], in_=bd[:, :])
        bo_c = const.tile([C, 1], F32, tag="bo")
        nc.sync.dma_start(out=bo_c[:, :], in_=bo_d[:, :])

        def emit_load(ci):
            if ci == 0:
                y_sb = yp.tile([C, D, 2, W], F32, tag="y")
                first_ld["y"] = y_sb
                return first_ld
            b, hp = CH[ci]
            h0 = 2 * hp
            x_sb = xp.tile([C, D, 2, W], F32, tag="x")
            nc.sync.dma_start(out=x_sb[:, :, :, :],
                              in_=x_d[b, :, :, h0:h0 + 2, :])
            # bf16 copy of x for the projections: gpsimd software-DGE
            # SBUF->SBUF DMA casts for free (64 big descriptors)
            x_bf = xp.tile([C, D, 2, W], BF16, tag="xbf")
            nc.scalar.activation(x_bf[:, :, :, :], x_sb[:, :, :, :],
                                 mybir.ActivationFunctionType.Copy)
            y_sb = yp.tile([C, D, 2, W], F32, tag="y")
            return {"x": x_sb, "xbf": x_bf, "y": y_sb,
                    "half": [None] * 4, "ci": ci}

        def emit_xb(st):
            # xb = x + bo (residual + out-proj bias), on the idle GPSIMD.
            # Emitted AFTER the next chunk's cast so the 5.8us Pool op never
            # blocks the cast on the in-order Pool queue.
            xb_sb = yp.tile([C, D, 2, W], F32, tag="xb")
            nc.gpsimd.tensor_scalar_add(xb_sb[:, :, :, :],
                                        st["x"][:, :, :, :], bo_c[:, :])
            st["xb"] = xb_sb

        def emit_proj_half(st, half):
            wb = 32 * half
            kg = kg_ps.tile([128, 256], F32, tag="kg")
            qg = qg_ps.tile([128, 256], F32, tag="qg")
            for r in range(4):
                rhs = mkap(st["xbf"], 0, C, wb + r, [[4, 8], [2 * W, D]])
                nc.tensor.matmul(kg[32 * r:32 * r + 32, :],
                                 wkT[:, :], rhs, start=True, stop=True,
                                 tile_position=(0, 32 * r))
                nc.tensor.matmul(qg[32 * r:32 * r + 32, :],
                                 wqT[:, :], rhs, start=True, stop=True,
                                 tile_position=(0, 32 * r))
            k_sb = sb.tile([128, 256], BF16, tag="k")
            q_sb = sb.tile([128, 256], BF16, tag="q")
            nc.scalar.activation(k_sb[:, :], kg[:, :],
                                 mybir.ActivationFunctionType.Identity,
                                 bias=bk_t[:, :])
            nc.scalar.activation(q_sb[:, :], qg[:, :],
                                 mybir.ActivationFunctionType.Identity,
                                 bias=bq_t[:, :])
            st["half"][half] = {"k": k_sb, "q": q_sb}

        def emit_vt(st, half):
            wb = 32 * half
            vt = vt_ps.tile([128, 256], F32, tag="vt")
            for r in range(4):
                rhs = mkap(st["xbf"], 0, C, wb + r, [[4, 8], [2 * W, D]])
                nc.tensor.matmul(vt[32 * r:32 * r + 32, :],
                                 wvT[:, :], rhs, start=True, stop=True,
                                 tile_position=(0, 32 * r))
            v_sb = sb.tile([128, 256], BF16, tag="v")
            nc.scalar.activation(v_sb[:, :], vt[:, :],
                                 mybir.ActivationFunctionType.Identity,
                                 bias=bv_t[:, :])
            vT_sb = sb.tile([128, 256], BF16, tag="vT")
            nc.vector.transpose(vT_sb[:, :], v_sb[:, :])
            st["half"][half]["vT"] = vT_sb

        def emit_scores(st, half):
            hh = st["half"][half]
            sc = sc_ps.tile([128, 256], F32, tag="sc")
            k_sb, q_sb = hh["k"], hh["q"]
            for u in range(8):
                for r in range(4):
                    nc.tensor.matmul(
                        sc[32 * r:32 * r + 32, 32 * u:32 * u + 32],
                        q_sb[32 * r:32 * r + 32, 32 * u:32 * u + 32],
                        k_sb[32 * r:32 * r + 32, 32 * u:32 * u + 32],
                        start=True, stop=True,
                        tile_position=(32 * r, 32 * r))
            e_sb = sb.tile([128, 256], BF16, tag="e")
            nc.scalar.activation(e_sb[:, :], sc[:, :],
                                 mybir.ActivationFunctionType.Exp,
                                 scale=INV_SQRT_S)
            den = sb.tile([128, 8], F32, tag="den")
            nc.vector.reduce_sum(
                out=den[:, :],
                in_=mkap(e_sb, 0, 128, 0, [[32, 8], [1, 32]]),
                axis=mybir.AxisListType.X)
            rcp = sb.tile([128, 8], F32, tag="rcp")
            nc.vector.reciprocal(rcp[:, :], den[:, :])
            aT_sb = sb.tile([128, 256], BF16, tag="aT")
            nc.vector.tensor_tensor(
                out=mkap(aT_sb, 0, 128, 0, [[32, 8], [1, 32]]),
                in0=mkap(e_sb, 0, 128, 0, [[32, 8], [1, 32]]),
                in1=mkap(rcp, 0, 128, 0, [[1, 8], [0, 32]]),
                op=mybir.AluOpType.mult)
            a_sb = sb.tile([128, 256], BF16, tag="a")
            nc.vector.transpose(a_sb[:, :], aT_sb[:, :])
            hh["a"] = a_sb

        def emit_att_out(st, half):
            hh = st["half"][half]
            a_sb, vT_sb = hh["a"], hh["vT"]
            og = og_ps.tile([128, 256], F32, tag="og")
            for u in range(8):
                for r in range(4):
                    nc.tensor.matmul(
                        og[32 * r:32 * r + 32, 32 * u:32 * u + 32],
                        vT_sb[32 * r:32 * r + 32, 32 * u:32 * u + 32],
                        a_sb[32 * r:32 * r + 32, 32 * u:32 * u + 32],
                        start=True, stop=True,
                        tile_position=(32 * r, 32 * r))
            o_sb = sb.tile([128, 256], BF16, tag="osb")
            nc.scalar.activation(o_sb[:, :], og[:, :],
                                 mybir.ActivationFunctionType.Copy)
            hh["o"] = o_sb

        def emit_outproj(st, half):
            wb = 32 * half
            o_sb = st["half"][half]["o"]
            # M=32 bh-split: a single M=64 matmul (32x64 PE tile) crashes
            # the device, so split the C dim across two 32x32-tile matmuls.
            # Two [C,256] tiles ping-pong across the 4 r-groups.
            for r in range(4):
                opt = op_ps.tile([C, 256], F32, tag=f"op{r % 2}")
                for bh in range(2):
                    nc.tensor.matmul(
                        opt[32 * bh:32 * bh + 32, :],
                        woT[32 * r:32 * r + 32, 32 * bh:32 * bh + 32],
                        o_sb[32 * r:32 * r + 32, :],
                        start=True, stop=True,
                        tile_position=(32 * r, 32 * bh))
                # residual: element (c, u, d): op free = 32u + d,
                # y free = 128d + wb + r + 4u
                in0 = mkap(opt, 0, C, 0, [[32, 8], [1, 32]])
                x_in = mkap(st["xb"], 0, C, wb + r, [[4, 8], [2 * W, D]])
                y_out = mkap(st["y"], 0, C, wb + r, [[4, 8], [2 * W, D]])
                nc.vector.tensor_tensor(out=y_out, in0=in0, in1=x_in,
                                        op=mybir.AluOpType.add)

        def emit_store(st):
            b, hp = CH[st["ci"]]
            h0 = 2 * hp
            nc.sync.dma_start(out=y_d[b, :, :, h0:h0 + 2, :],
                               in_=st["y"][:, :, :, :])

        cur = None
        nxt = emit_load(0)
        for ci in range(len(CH) + 1):
            if ci + 1 < len(CH):
                pre = emit_load(ci + 1)  # prefetch: dma + cast ahead of xb
            else:
                pre = None
            if nxt is not None:
                emit_xb(nxt)
            for half in range(4):
                if nxt is not None:
                    emit_proj_half(nxt, half)
                if cur is not None:
                    emit_scores(cur, half)
                if nxt is not None:
                    emit_vt(nxt, half)
                if cur is not None:
                    if half >= 2:
                        emit_att_out(cur, half - 2)
                    if half >= 3:
                        emit_outproj(cur, half - 3)
            if cur is not None:
                emit_att_out(cur, 2)
                emit_att_out(cur, 3)
                emit_outproj(cur, 1)
                emit_outproj(cur, 2)
                emit_outproj(cur, 3)
                emit_store(cur)
            cur = nxt
            nxt = pre

    nc.finalize()
    return nc


_NC_CACHE = {}


def get_nc(key=(2, 1, 3, 1)):
    if key not in _NC_CACHE:
        _NC_CACHE[key] = build_program(*key)
    return _NC_CACHE[key]


def make_in_maps(x, Wk, bk, Wq, bq, Wv, bv, Wo, bo):
    x = np.ascontiguousarray(np.asarray(x, dtype=np.float32))
    f = np.float32
    rep4 = lambda v: np.tile(np.asarray(v, f).reshape(-1), 4)[:, None]
    consts = {
        "wkT": np.ascontiguousarray(np.asarray(Wk, f).T),
        "wqT": np.ascontiguousarray(np.asarray(Wq, f).T),
        "wvT": np.ascontiguousarray(np.asarray(Wv, f).T),
        "woTr": np.ascontiguousarray(np.tile(np.asarray(Wo, f).T, (4, 1))),
        "bkr": np.ascontiguousarray(rep4(bk)),
        "bqr": np.ascontiguousarray(rep4(bq)),
        "bvr": np.ascontiguousarray(rep4(bv)),
        "boc": np.ascontiguousarray(np.asarray(bo, f)[:, None]),
    }
    in_maps = []
    for i in range(NCORES):
        m = {"x": np.ascontiguousarray(x[:, :, :, i * HS:(i + 1) * HS, :])}
        m.update(consts)
        in_maps.append(m)
    return in_maps


def gather(results):
    out = np.empty((B, C, D, H, W), dtype=np.float32)
    for i in range(NCORES):
        out[:, :, :, i * HS:(i + 1) * HS, :] = results[i]["y"]
    return out


def kernel(x, Wk, bk, Wq, bq, Wv, bv, Wo, bo):
    nc = get_nc()
    in_maps = make_in_maps(x, Wk, bk, Wq, bq, Wv, bv, Wo, bo)
    res = run_bass_kernel_spmd(nc, in_maps, core_ids=list(range(NCORES)))
    return gather(res.results)


# revision 49
# speedup vs baseline: 3.9197x; 1.0836x over previous
"""Trainium2 Bass kernel for 3D conv-attention layer.

Reference (per (b,h,w) "site", D=32 positions, S=32 features):
  k,q,v = 1x1 conv of x [B,C,D,H,W] -> [B,S,D,H,W]
  scoresT[j,i] = sum_s q[s,j] k[s,i] / sqrt(S)   (per site)
  aT = softmax over i  (free dim of scoresT)
  o[s,j] = sum_i v[s,i] a[i,j];   y = x + Wo @ o + bo

Sharding: data-parallel over H across 8 cores (HS=8 rows each).

Per-core v3 design (cost-model driven):
  - Chunks of (b, h-pair): x/y DMA'd as [C, D, 2, W] so contiguous runs are
    2*W*4 = 512B (avoids the <512B 2x DMA latency penalty).
  - Software pipeline at chunk level: proj halves of chunk N interleave with
    attention halves of chunk N-1 in emission order, so the PE always has
    independent work while a half's softmax chain (ACT/DVE) is in flight.
  - All matmuls bf16 (1 PE cycle/row) except nothing in fp32 (4 cycles/row).
    x cast to bf16 for free via gpsimd software-DGE SBUF->SBUF DMA.
  - Per-site 32x32 attention matmuls into shared PSUM tiles so softmax runs
    as single whole-tile instructions ([128,256] per half of 32 sites).
  - PSUM: 8 banks = kq[128,512]x1 + vg[128,256]x1 + attn(sc|og)[128,512]x2
    + (op0,op1)[64,512]x2.
  - Residual y = op + (x+bo) as two 3-free-dim DVE adds per half; x+bo
    computed in place on the otherwise-idle GPSIMD (Pool).
  - Output y DMA'd from the ACT HWDGE queue so it never blocks input loads
    on the SP queue.
"""

import math
from contextlib import ExitStack

import numpy as np

import concourse.bass as bass
import concourse.mybir as mybir
from concourse import bacc
import concourse.tile as tile
from concourse.bass_utils import run_bass_kernel_spmd

B, C, D, H, W = 4, 64, 32, 64, 64
S = C // 2  # 32
NCORES = 8
HS = H // NCORES
HP = HS // 2  # h-pairs (chunks) per (b,) => 4
F32 = mybir.dt.float32
FR = mybir.dt.float32r
BF16 = mybir.dt.bfloat16

INV_SQRT_S = 1.0 / math.sqrt(S)


def mkap(base, part0, pcount, foff, fdims):
    """AP at partition block [part0, part0+pcount) of a tile, free offset foff,
    free dims [(step, count), ...] in the tile's flat free space."""
    full = base[...] if not isinstance(base, bass.AP) else base
    pstride = full.ap[0][0]
    return bass.AP(tensor=full.tensor,
                   offset=full.offset + part0 * pstride + foff,
                   ap=[[pstride, pcount]] + [list(d) for d in fdims])


def build_program(kq_bufs=2, vt_bufs=1, at_bufs=3, op_bufs=1):
    nc = bacc.Bacc()
    x_d = nc.declare_dram_parameter("x", [B, C, D, HS, W], F32, isOutput=False)
    wk_d = nc.declare_dram_parameter("wkT", [C, S], F32, isOutput=False)
    wq_d = nc.declare_dram_parameter("wqT", [C, S], F32, isOutput=False)
    wv_d = nc.declare_dram_parameter("wvT", [C, S], F32, isOutput=False)
    wo_d = nc.declare_dram_parameter("woTr", [4 * S, C], F32, isOutput=False)
    bk_d = nc.declare_dram_parameter("bkr", [128, 1], F32, isOutput=False)
    bq_d = nc.declare_dram_parameter("bqr", [128, 1], F32, isOutput=False)
    bv_d = nc.declare_dram_parameter("bvr", [128, 1], F32, isOutput=False)
    bo_d = nc.declare_dram_parameter("boc", [C, 1], F32, isOutput=False)
    y_d = nc.declare_dram_parameter("y", [B, C, D, HS, W], F32, isOutput=True)

    with tile.TileContext(nc) as tc, ExitStack() as ctx:
        const = ctx.enter_context(tc.tile_pool(name="const", bufs=1))
        xp = ctx.enter_context(tc.tile_pool(name="xp", bufs=3))
        yp = ctx.enter_context(tc.tile_pool(name="yp", bufs=3))
        kg_ps = ctx.enter_context(tc.tile_pool(name="kg_ps", bufs=1, space="PSUM"))
        qg_ps = ctx.enter_context(tc.tile_pool(name="qg_ps", bufs=1, space="PSUM"))
        vt_ps = ctx.enter_context(tc.tile_pool(name="vt_ps", bufs=1, space="PSUM"))
        sc_ps = ctx.enter_context(tc.tile_pool(name="sc_ps", bufs=2, space="PSUM"))
        og_ps = ctx.enter_context(tc.tile_pool(name="og_ps", bufs=1, space="PSUM"))
        op_ps = ctx.enter_context(tc.tile_pool(name="op_ps", bufs=1, space="PSUM"))
        sb = ctx.enter_context(tc.tile_pool(name="sb", bufs=6))

        CH = [(b, hp) for b in range(B) for hp in range(HP)]

        first_ld = {}

        def emit_first_load():
            b, hp = CH[0]
            h0 = 2 * hp
            x_sb = xp.tile([C, D, 2, W], F32, tag="x")
            nc.sync.dma_start(out=x_sb[:, :, :, :],
                              in_=x_d[b, :, :, h0:h0 + 2, :])
            x_bf = xp.tile([C, D, 2, W], BF16, tag="xbf")
            nc.gpsimd.dma_start(out=x_bf[:, :, :, :], in_=x_sb[:, :, :, :])
            first_ld.update({"x": x_sb, "xbf": x_bf,
                             "half": [None] * 4, "ci": 0})

        emit_first_load()

        # ---- constants (weights cast to bf16 on DVE once) ----
        wkT_s = const.tile([C, S], F32, tag="wks")
        wqT_s = const.tile([C, S], F32, tag="wqs")
        wvT_s = const.tile([C, S], F32, tag="wvs")
        wkT = const.tile([C, S], BF16, tag="wk")
        wqT = const.tile([C, S], BF16, tag="wq")
        wvT = const.tile([C, S], BF16, tag="wv")
        for wt, ws, wd in ((wkT, wkT_s, wk_d), (wqT, wqT_s, wq_d),
                           (wvT, wvT_s, wv_d)):
            nc.sync.dma_start(out=ws[:, :], in_=wd[:, :])
            nc.vector.tensor_copy(out=wt[:, :], in_=ws[:, :])
        # out-proj weights 4x-tiled on partitions (lhsT base must match the
        # rhs partition base 32r), cast to bf16
        woT_s = const.tile([4 * S, C], F32, tag="wos")
        woT = const.tile([4 * S, C], BF16, tag="wo")
        nc.sync.dma_start(out=woT_s[:, :], in_=wo_d[:, :])
        nc.vector.tensor_copy(out=woT[:, :], in_=woT_s[:, :])
        bk_t = const.tile([128, 1], F32, tag="bk")
        bq_t = const.tile([128, 1], F32, tag="bq")
        bv_t = const.tile([128, 1], F32, tag="bv")
        for bt, bd in ((bk_t, bk_d), (bq_t, bq_d), (bv_t, bv_d)):
            nc.sync.dma_start(out=bt[:, :], in_=bd[:, :])
        bo_c = const.tile([C, 1], F32, tag="bo")
        nc.sync.dma_start(out=bo_c[:, :], in_=bo_d[:, :])

        def emit_load(ci):
            if ci == 0:
                y_sb = yp.tile([C, D, 2, W], F32, tag="y")
                first_ld["y"] = y_sb
                return first_ld
            b, hp = CH[ci]
            h0 = 2 * hp
            x_sb = xp.tile([C, D, 2, W], F32, tag="x")
            nc.sync.dma_start(out=x_sb[:, :, :, :],
                              in_=x_d[b, :, :, h0:h0 + 2, :])
            # bf16 copy of x for the projections: gpsimd software-DGE
            # SBUF->SBUF DMA casts for free (64 big descriptors)
            x_bf = xp.tile([C, D, 2, W], BF16, tag="xbf")
            nc.gpsimd.dma_start(out=x_bf[:, :, :, :], in_=x_sb[:, :, :, :])
            y_sb = yp.tile([C, D, 2, W], F32, tag="y")
            return {"x": x_sb, "xbf": x_bf, "y": y_sb,
                    "half": [None] * 4, "ci": ci}

        def emit_xb(st):
            # xb = x + bo (residual + out-proj bias), on the idle GPSIMD.
            # Emitted AFTER the next chunk's cast so the 5.8us Pool op never
            # blocks the cast on the in-order Pool queue.
            xb_sb = yp.tile([C, D, 2, W], F32, tag="xb")
            nc.gpsimd.tensor_scalar_add(xb_sb[:, :, :, :],
                                        st["x"][:, :, :, :], bo_c[:, :])
            st["xb"] = xb_sb

        def emit_proj_half(st, half):
            wb = 32 * half
            kg = kg_ps.tile([128, 256], F32, tag="kg")
            qg = qg_ps.tile([128, 256], F32, tag="qg")
            for r in range(4):
                rhs = mkap(st["xbf"], 0, C, wb + r, [[4, 8], [2 * W, D]])
                nc.tensor.matmul(kg[32 * r:32 * r + 32, :],
                                 wkT[:, :], rhs, start=True, stop=True,
                                 tile_position=(0, 32 * r))
                nc.tensor.matmul(qg[32 * r:32 * r + 32, :],
                                 wqT[:, :], rhs, start=True, stop=True,
                                 tile_position=(0, 32 * r))
            k_sb = sb.tile([128, 256], BF16, tag="k")
            q_sb = sb.tile([128, 256], BF16, tag="q")
            nc.scalar.activation(k_sb[:, :], kg[:, :],
                                 mybir.ActivationFunctionType.Identity,
                                 bias=bk_t[:, :])
            nc.scalar.activation(q_sb[:, :], qg[:, :],
                                 mybir.ActivationFunctionType.Identity,
                                 bias=bq_t[:, :])
            st["half"][half] = {"k": k_sb, "q": q_sb}

        def emit_vt(st, half):
            wb = 32 * half
            vt = vt_ps.tile([128, 256], F32, tag="vt")
            for r in range(4):
                rhs = mkap(st["xbf"], 0, C, wb + r, [[4, 8], [2 * W, D]])
                nc.tensor.matmul(vt[32 * r:32 * r + 32, :],
                                 wvT[:, :], rhs, start=True, stop=True,
                                 tile_position=(0, 32 * r))
            v_sb = sb.tile([128, 256], BF16, tag="v")
            nc.scalar.activation(v_sb[:, :], vt[:, :],
                                 mybir.ActivationFunctionType.Identity,
                                 bias=bv_t[:, :])
            vT_sb = sb.tile([128, 256], BF16, tag="vT")
            nc.vector.transpose(vT_sb[:, :], v_sb[:, :])
            st["half"][half]["vT"] = vT_sb

        def emit_scores(st, half):
            hh = st["half"][half]
            sc = sc_ps.tile([128, 256], F32, tag="sc")
            k_sb, q_sb = hh["k"], hh["q"]
            for u in range(8):
                for r in range(4):
                    nc.tensor.matmul(
                        sc[32 * r:32 * r + 32, 32 * u:32 * u + 32],
                        q_sb[32 * r:32 * r + 32, 32 * u:32 * u + 32],
                        k_sb[32 * r:32 * r + 32, 32 * u:32 * u + 32],
                        start=True, stop=True,
                        tile_position=(32 * r, 32 * r))
            e_sb = sb.tile([128, 256], BF16, tag="e")
            nc.scalar.activation(e_sb[:, :], sc[:, :],
                                 mybir.ActivationFunctionType.Exp,
                                 scale=INV_SQRT_S)
            den = sb.tile([128, 8], F32, tag="den")
            nc.vector.reduce_sum(
                out=den[:, :],
                in_=mkap(e_sb, 0, 128, 0, [[32, 8], [1, 32]]),
                axis=mybir.AxisListType.X)
            rcp = sb.tile([128, 8], F32, tag="rcp")
            nc.vector.reciprocal(rcp[:, :], den[:, :])
            aT_sb = sb.tile([128, 256], BF16, tag="aT")
            nc.vector.tensor_tensor(
                out=mkap(aT_sb, 0, 128, 0, [[32, 8], [1, 32]]),
                in0=mkap(e_sb, 0, 128, 0, [[32, 8], [1, 32]]),
                in1=mkap(rcp, 0, 128, 0, [[1, 8], [0, 32]]),
                op=mybir.AluOpType.mult)
            a_sb = sb.tile([128, 256], BF16, tag="a")
            nc.vector.transpose(a_sb[:, :], aT_sb[:, :])
            hh["a"] = a_sb

        def emit_att_out(st, half):
            hh = st["half"][half]
            a_sb, vT_sb = hh["a"], hh["vT"]
            og = og_ps.tile([128, 256], F32, tag="og")
            for u in range(8):
                for r in range(4):
                    nc.tensor.matmul(
                        og[32 * r:32 * r + 32, 32 * u:32 * u + 32],
                        vT_sb[32 * r:32 * r + 32, 32 * u:32 * u + 32],
                        a_sb[32 * r:32 * r + 32, 32 * u:32 * u + 32],
                        start=True, stop=True,
                        tile_position=(32 * r, 32 * r))
            o_sb = sb.tile([128, 256], BF16, tag="osb")
            nc.scalar.activation(o_sb[:, :], og[:, :],
                                 mybir.ActivationFunctionType.Copy)
            hh["o"] = o_sb

        def emit_outproj(st, half):
            wb = 32 * half
            o_sb = st["half"][half]["o"]
            # M=32 bh-split: a single M=64 matmul (32x64 PE tile) crashes
            # the device, so split the C dim across two 32x32-tile matmuls.
            # Two [C,256] tiles ping-pong across the 4 r-groups.
            for r in range(4):
                opt = op_ps.tile([C, 256], F32, tag=f"op{r % 2}")
                for bh in range(2):
                    nc.tensor.matmul(
                        opt[32 * bh:32 * bh + 32, :],
                        woT[32 * r:32 * r + 32, 32 * bh:32 * bh + 32],
                        o_sb[32 * r:32 * r + 32, :],
                        start=True, stop=True,
                        tile_position=(32 * r, 32 * bh))
                # residual: element (c, u, d): op free = 32u + d,
                # y free = 128d + wb + r + 4u
                in0 = mkap(opt, 0, C, 0, [[32, 8], [1, 32]])
                x_in = mkap(st["xb"], 0, C, wb + r, [[4, 8], [2 * W, D]])
                y_out = mkap(st["y"], 0, C, wb + r, [[4, 8], [2 * W, D]])
                nc.vector.tensor_tensor(out=y_out, in0=in0, in1=x_in,
                                        op=mybir.AluOpType.add)

        def emit_store(st):
            b, hp = CH[st["ci"]]
            h0 = 2 * hp
            nc.sync.dma_start(out=y_d[b, :, :, h0:h0 + 2, :],
                               in_=st["y"][:, :, :, :])

        cur = None
        nxt = emit_load(0)
        for ci in range(len(CH) + 1):
            if ci + 1 < len(CH):
                pre = emit_load(ci + 1)  # prefetch: dma + cast ahead of xb
            else:
                pre = None
            if nxt is not None:
                emit_xb(nxt)
            for half in range(4):
                if nxt is not None:
                    emit_proj_half(nxt, half)
                if cur is not None:
                    emit_scores(cur, half)
                if nxt is not None:
                    emit_vt(nxt, half)
                if cur is not None:
                    if half >= 2:
                        emit_att_out(cur, half - 2)
                    if half >= 3:
                        emit_outproj(cur, half - 3)
            if cur is not None:
                emit_att_out(cur, 2)
                emit_att_out(cur, 3)
                emit_outproj(cur, 1)
                emit_outproj(cur, 2)
                emit_outproj(cur, 3)
                emit_store(cur)
            cur = nxt
            nxt = pre

    nc.finalize()
    return nc


_NC_CACHE = {}


def get_nc(key=(2, 1, 3, 1)):
    if key not in _NC_CACHE:
        _NC_CACHE[key] = build_program(*key)
    return _NC_CACHE[key]


def make_in_maps(x, Wk, bk, Wq, bq, Wv, bv, Wo, bo):
    x = np.ascontiguousarray(np.asarray(x, dtype=np.float32))
    f = np.float32
    rep4 = lambda v: np.tile(np.asarray(v, f).reshape(-1), 4)[:, None]
    consts = {
        "wkT": np.ascontiguousarray(np.asarray(Wk, f).T),
        "wqT": np.ascontiguousarray(np.asarray(Wq, f).T),
        "wvT": np.ascontiguousarray(np.asarray(Wv, f).T),
        "woTr": np.ascontiguousarray(np.tile(np.asarray(Wo, f).T, (4, 1))),
        "bkr": np.ascontiguousarray(rep4(bk)),
        "bqr": np.ascontiguousarray(rep4(bq)),
        "bvr": np.ascontiguousarray(rep4(bv)),
        "boc": np.ascontiguousarray(np.asarray(bo, f)[:, None]),
    }
    in_maps = []
    for i in range(NCORES):
        m = {"x": np.ascontiguousarray(x[:, :, :, i * HS:(i + 1) * HS, :])}
        m.update(consts)
        in_maps.append(m)
    return in_maps


def gather(results):
    out = np.empty((B, C, D, H, W), dtype=np.float32)
    for i in range(NCORES):
        out[:, :, :, i * HS:(i + 1) * HS, :] = results[i]["y"]
    return out


def kernel(x, Wk, bk, Wq, bq, Wv, bv, Wo, bo):
    nc = get_nc()
    in_maps = make_in_maps(x, Wk, bk, Wq, bq, Wv, bv, Wo, bo)
    res = run_bass_kernel_spmd(nc, in_maps, core_ids=list(range(NCORES)))
    return gather(res.results)
